# revision 7
# baseline (speedup 1.0000x reference)
"""Trainium2 Bass kernel for nn_DecoderRNN (attention LSTM decoder).

Strategy (8 NeuronCores, SPMD):
  - d-sliced / gates-sliced model parallelism for the 21-step recurrence:
    core j owns d-slice ds_j = [64j, 64j+64) of the attention/hidden dim and
    the aligned gates slice gs_j = {g*512 + 64j + k} (same 64 rows of each of
    the i/f/g/o gate blocks), plus vocab slice vs_j = [1500j, 1500j+1500).
  - The per-step context->gates matmul is hoisted via the "G-trick":
    W_ic @ context = sum_n softmax_n * (W_ic @ feat_n), with
    G = features @ W_ic.T precomputed once on device (PE, bf16).
  - Attention scores are reduced per-core over the local d-slice and combined
    with an AllReduce; hidden-state slices are recombined each step with a
    second AllReduce (one-hot positioned via a selector matmul).
  - The big fc head (V=12000) runs vocab-sharded, interleaved with the
    recurrence on the PE.

Host-side work is limited to slicing / transposition / dtype prep of inputs
(layout prep for the per-core in_maps) and concatenation of outputs.
"""

import numpy as np
import sys

sys.path.insert(0, "/opt/trn_rl_repo")

import concourse.bass as bass  # noqa: E402
import concourse.bacc as bacc  # noqa: E402
import concourse.mybir as mybir  # noqa: E402
import concourse.tile as tile  # noqa: E402
from concourse import masks  # noqa: E402
from concourse.bass_utils import run_bass_kernel_spmd  # noqa: E402

F32 = mybir.dt.float32
BF16 = mybir.dt.bfloat16
U16 = mybir.dt.uint16
AF = mybir.ActivationFunctionType
OP = mybir.AluOpType

B = 128
N = 49
E = 2048
EMB = 512
D = 512
V = 12000
NC = 8
DS = D // NC          # 64   d-slice per core
GS = 4 * DS           # 256  gates slice per core (64 rows of each gate)
VS = V // NC          # 1500 vocab slice per core
EK = E // 128         # 16   e chunks
RG = [list(range(NC))]

_CACHE = {}


def _bc(v, rows=128):
    """broadcast a 1-d vector to [rows, len(v)]"""
    return np.ascontiguousarray(np.broadcast_to(np.asarray(v)[None, :], (rows, len(v))))


def _bf(x):
    import ml_dtypes
    return np.ascontiguousarray(np.asarray(x)).astype(ml_dtypes.bfloat16)


def build_program(T):
    nc = bacc.Bacc("TRN2", target_bir_lowering=False, debug=False, num_devices=NC)

    # ---------------- I/O declarations (per-core payloads) ----------------
    dt = nc.dram_tensor
    featT_d = dt("featT", [E, N, B], BF16, kind="ExternalInput")
    feat_en_d = dt("feat_en", [B, 2 * 128, N], BF16, kind="ExternalInput")
    embT4_d = dt("embT4", [B, 4, T, B], BF16, kind="ExternalInput")
    rhs_gfp_d = dt("rhs_gfp", [B, EK, GS + DS], BF16, kind="ExternalInput")
    WieT_d = dt("WieT", [B, 4, GS], BF16, kind="ExternalInput")
    WhhT_d = dt("WhhT", [B, 4, GS], BF16, kind="ExternalInput")
    UaT_d = dt("UaT", [B, 4, DS], BF16, kind="ExternalInput")
    fcT_d = dt("fcT", [B, 4, VS], BF16, kind="ExternalInput")
    ihicT_d = dt("ihicT", [B, 2, 2 * D], BF16, kind="ExternalInput")
    ihicb_d = dt("ihicb", [B, 2 * D], F32, kind="ExternalInput")
    biasg_d = dt("biasg", [B, GS], F32, kind="ExternalInput")
    wuab_d = dt("wuab", [B, DS], F32, kind="ExternalInput")
    vab_d = dt("vab", [B, DS], BF16, kind="ExternalInput")
    sel_d = dt("sel", [DS, D], BF16, kind="ExternalInput")
    c0idx_d = dt("c0idx", [B, DS // 16], U16, kind="ExternalInput")
    fcb_d = dt("fcb", [B, VS], F32, kind="ExternalInput")

    att_d = dt("att", [B, T, N], F32, kind="ExternalOutput")
    preds_d = dt("preds", [B, T, VS], F32, kind="ExternalOutput")

    VCH = [(0, 512), (512, 512), (1024, VS - 1024)]  # fc column chunks

    with tile.TileContext(nc) as tc:
        with (
            tc.tile_pool(name="const", bufs=1) as cpool,
            tc.tile_pool(name="state", bufs=1) as spool,
            tc.tile_pool(name="work", bufs=2) as wpool,
            tc.tile_pool(name="fcw", bufs=2) as fcwpool,
            tc.tile_pool(name="dram", bufs=4, space="DRAM") as dram,
        ):
            # ------------- resident constants / weights -------------
            identb = cpool.tile([128, 128], BF16)
            masks.make_identity(nc, identb[:])
            identf = cpool.tile([128, 128], F32)
            masks.make_identity(nc, identf[:])

            rhs_gfp = cpool.tile([B, EK, GS + DS], BF16)
            nc.sync.dma_start(rhs_gfp[:], rhs_gfp_d[:])
            WieT = cpool.tile([B, 4, GS], BF16)
            nc.sync.dma_start(WieT[:], WieT_d[:])
            WhhT = cpool.tile([B, 4, GS], BF16)
            nc.sync.dma_start(WhhT[:], WhhT_d[:])
            UaT = cpool.tile([B, 4, DS], BF16)
            nc.sync.dma_start(UaT[:], UaT_d[:])
            fcT = cpool.tile([B, 4, VS], BF16)
            nc.sync.dma_start(fcT[:], fcT_d[:])
            biasg = cpool.tile([B, GS], F32)
            nc.sync.dma_start(biasg[:], biasg_d[:])
            wuab = cpool.tile([B, DS], F32)
            nc.sync.dma_start(wuab[:], wuab_d[:])
            vab = cpool.tile([B, DS], BF16)
            nc.sync.dma_start(vab[:], vab_d[:])
            sel = cpool.tile([DS, D], BF16)
            nc.sync.dma_start(sel[:], sel_d[:])
            fcb = cpool.tile([B, VS], F32)
            nc.sync.dma_start(fcb[:], fcb_d[:])

            # ------------- persistent state / precomputed activations -------------
            G = spool.tile([B, N, GS], BF16)
            fpT = spool.tile([B, N, DS], BF16)
            gates_e = spool.tile([B, T, GS], BF16)
            c_t = spool.tile([B, DS], F32)
            h2full = spool.tile([B, D], BF16)
            hT = spool.tile([B, 4, B], BF16)

            # ================= P0: h0/c0 =================
            with (
                tc.tile_pool(name="p0", bufs=1) as p0,
                tc.tile_pool(name="p0ps", bufs=1, space="PSUM") as p0ps,
                tc.tile_pool(name="p0ps2", bufs=2, space="PSUM") as p0ps2,
            ):
                feat_en = p0.tile([B, 2 * 128, N], BF16)
                nc.sync.dma_start(feat_en[:], feat_en_d[:])
                ihicT = p0.tile([B, 2, 2 * D], BF16)
                nc.sync.dma_start(ihicT[:], ihicT_d[:])
                ihicb = p0.tile([B, 2 * D], F32)
                nc.sync.dma_start(ihicb[:], ihicb_d[:])

                sumfeat = p0.tile([B, 2 * 128], F32)
                nc.vector.tensor_reduce(
                    sumfeat[:], feat_en[:], axis=mybir.AxisListType.X, op=OP.add
                )
                sfT = p0.tile([B, 2, 128], BF16)
                for k in range(2):
                    tp = p0ps2.tile([128, 128], F32, tag="tp")
                    nc.tensor.transpose(tp[:], sumfeat[:, k * 128:(k + 1) * 128], identf[:])
                    nc.vector.tensor_copy(sfT[:, k, :], tp[:])
                ps_hc = p0ps.tile([B, 2 * D], F32)
                for k in range(2):
                    for hh in range(2):
                        nc.tensor.matmul(
                            ps_hc[:, hh * D:(hh + 1) * D],
                            sfT[:, k, :],
                            ihicT[:, k, hh * D:(hh + 1) * D],
                            start=(k == 0),
                            stop=(k == 1),
                        )
                hcp = p0.tile([B, 2 * D], F32)
                nc.vector.tensor_copy(hcp[:], ps_hc[:])
                bi = dram.tile([B, 2 * D], F32, tag="hc")
                bo = dram.tile([B, 2 * D], F32, tag="hc")
                nc.sync.dma_start(bi[:], hcp[:])
                nc.gpsimd.collective_compute(
                    "AllReduce", OP.add, replica_groups=RG, ins=[bi[:]], outs=[bo[:]]
                )
                hc = p0.tile([B, 2 * D], F32)
                nc.sync.dma_start(hc[:], bo[:])
                nc.vector.tensor_add(hc[:], hc[:], ihicb[:])
                nc.vector.tensor_copy(h2full[:], hc[:, 0:D])
                c0i = p0.tile([B, DS // 16], U16)
                nc.sync.dma_start(c0i[:], c0idx_d[:])
                nc.gpsimd.indirect_copy(c_t[:], hc[:], c0i[:], True)

            # ================= P0: G + f_proj (fused featT stream) =================
            with (
                tc.tile_pool(name="gstream", bufs=6) as gpool,
                tc.tile_pool(name="gps", bufs=2, space="PSUM") as gps,
            ):
                for n in range(N):
                    ps = gps.tile([B, GS + DS], F32, tag="gfp")
                    for k in range(EK):
                        ft = gpool.tile([128, B], BF16, tag="ft")
                        nc.sync.dma_start(ft[:], featT_d[k * 128:(k + 1) * 128, n, :])
                        nc.tensor.matmul(
                            ps[:], ft[:], rhs_gfp[:, k, :],
                            start=(k == 0), stop=(k == EK - 1),
                        )
                    nc.scalar.copy(G[:, n, :], ps[:, 0:GS])
                    nc.vector.tensor_copy(fpT[:, n, :], ps[:, GS:GS + DS])

            # ================= P0: gates_e (embed part + biases) =================
            with (
                tc.tile_pool(name="geps", bufs=2, space="PSUM") as geps,
                tc.tile_pool(name="gepool", bufs=3) as gepool,
            ):
                for t in range(T):
                    et = gepool.tile([B, 4, B], BF16, tag="et")
                    nc.sync.dma_start(et[:], embT4_d[:, :, t, :])
                    ps = geps.tile([B, GS], F32, tag="ge")
                    for k in range(4):
                        nc.tensor.matmul(
                            ps[:], et[:, k, :], WieT[:, k, :],
                            start=(k == 0), stop=(k == 3),
                        )
                    nc.vector.tensor_add(gates_e[:, t, :], ps[:], biasg[:])

            # ================= recurrence =================
            with (
                tc.tile_pool(name="ps_tp", bufs=2, space="PSUM") as ps_tp,
                tc.tile_pool(name="ps_hp", bufs=1, space="PSUM") as ps_hp,
                tc.tile_pool(name="ps_g", bufs=1, space="PSUM") as ps_g,
                tc.tile_pool(name="ps_ht", bufs=1, space="PSUM") as ps_ht,
                tc.tile_pool(name="ps_sel", bufs=1, space="PSUM") as ps_sel,
                tc.tile_pool(name="ps_fc", bufs=2, space="PSUM") as ps_fc,
            ):
                def transposes_and_fc(t_prev):
                    # hT <- transpose(h2full);  preds[t_prev] <- fc(hT) if t_prev >= 0
                    for k in range(4):
                        tp = ps_tp.tile([128, 128], BF16, tag="tp")
                        nc.tensor.transpose(
                            tp[:], h2full[:, k * 128:(k + 1) * 128], identb[:]
                        )
                        nc.vector.tensor_copy(hT[:, k, :], tp[:])
                    if t_prev >= 0:
                        for (c0, cw) in VCH:
                            ps = ps_fc.tile([B, 512], F32, tag="fc")
                            for k in range(4):
                                nc.tensor.matmul(
                                    ps[:, 0:cw], hT[:, k, :], fcT[:, k, c0:c0 + cw],
                                    start=(k == 0), stop=(k == 3),
                                )
                            ot = wpool.tile([B, 512], F32, tag="fcout")
                            nc.vector.tensor_add(ot[:, 0:cw], ps[:, 0:cw], fcb[:, c0:c0 + cw])
                            nc.sync.dma_start(preds_d[:, t_prev, c0:c0 + cw], ot[:, 0:cw])

                for t in range(T):
                    transposes_and_fc(t - 1)

                    # ---- attention ----
                    hp_ps = ps_hp.tile([B, DS], F32, tag="hp")
                    for k in range(4):
                        nc.tensor.matmul(
                            hp_ps[:], hT[:, k, :], UaT[:, k, :],
                            start=(k == 0), stop=(k == 3),
                        )
                    hp = wpool.tile([B, DS], BF16, tag="hp_sb")
                    nc.vector.tensor_add(hp[:], hp_ps[:], wuab[:])

                    comb_in = wpool.tile([B, N, DS], BF16, tag="compin")
                    hp_b = hp[:].rearrange("p (o d) -> p o d", o=1).broadcast_to((B, N, DS))
                    nc.vector.tensor_add(comb_in[:], fpT[:], hp_b)
                    comb = wpool.tile([B, N, DS], BF16, tag="comb")
                    nc.scalar.activation(comb[:], comb_in[:], AF.Tanh)
                    sm = wpool.tile([B, N, DS], BF16, tag="sm")
                    vab_b = vab[:].rearrange("p (o d) -> p o d", o=1).broadcast_to((B, N, DS))
                    nc.vector.tensor_mul(sm[:], comb[:], vab_b)
                    sp = wpool.tile([B, N], F32, tag="sp")
                    nc.vector.tensor_reduce(sp[:], sm[:], axis=mybir.AxisListType.X, op=OP.add)

                    # ---- scores AllReduce ----
                    sbi = dram.tile([B, N], F32, tag="sc_i")
                    sbo = dram.tile([B, N], F32, tag="sc_o")
                    nc.sync.dma_start(sbi[:], sp[:])
                    nc.gpsimd.collective_compute(
                        "AllReduce", OP.add, replica_groups=RG, ins=[sbi[:]], outs=[sbo[:]]
                    )
                    sfull = wpool.tile([B, N], F32, tag="sfull")
                    nc.sync.dma_start(sfull[:], sbo[:])

                    # ---- softmax ----
                    nmax = wpool.tile([B, 1], F32, tag="nmax")
                    nc.vector.tensor_reduce(
                        nmax[:], sfull[:], axis=mybir.AxisListType.X, op=OP.max, negate=True
                    )
                    es = wpool.tile([B, N], F32, tag="es")
                    sume = wpool.tile([B, 1], F32, tag="sume")
                    nc.scalar.activation(
                        es[:], sfull[:], AF.Exp, bias=nmax[:], scale=1.0, accum_out=sume[:]
                    )
                    rs = wpool.tile([B, 1], F32, tag="rs")
                    nc.vector.reciprocal(rs[:], sume[:])
                    s_f = wpool.tile([B, N], F32, tag="s_f")
                    nc.vector.tensor_scalar(s_f[:], es[:], rs[:], None, OP.mult)
                    nc.sync.dma_start(att_d[:, t, :], s_f[:])
                    s_b = wpool.tile([B, N], BF16, tag="s_b")
                    nc.scalar.copy(s_b[:], s_f[:])

                    # ---- diagonals (gpsimd) ----
                    NCH = [(0, 13), (13, 13), (26, 13), (39, 10)]
                    dgs = []
                    for (n0, nw) in NCH:
                        dg = wpool.tile([B, nw, 128], BF16, tag=f"dg{n0}")
                        sb_b = (
                            s_b[:, n0:n0 + nw]
                            .rearrange("p (n o) -> p n o", o=1)
                            .broadcast_to((B, nw, 128))
                        )
                        nc.gpsimd.affine_select(
                            out=dg[:], in_=sb_b, compare_op=OP.is_equal, fill=0.0,
                            base=0, pattern=[[0, nw], [-1, 128]], channel_multiplier=1,
                        )
                        dgs.append((n0, nw, dg))

                    # ---- gates ----
                    g_ps = ps_g.tile([B, GS], F32, tag="g")
                    for k in range(4):
                        nc.tensor.matmul(
                            g_ps[:], hT[:, k, :], WhhT[:, k, :],
                            start=(k == 0), stop=False,
                        )
                    nc.tensor.matmul(
                        g_ps[:], identb[:], gates_e[:, t, :], start=False, stop=False
                    )
                    for (n0, nw, dg) in dgs:
                        for ni in range(nw):
                            n = n0 + ni
                            nc.tensor.matmul(
                                g_ps[:], dg[:, ni, :], G[:, n, :],
                                start=False, stop=(n == N - 1),
                            )

                    # ---- LSTM pointwise ----
                    t_if = wpool.tile([B, 2 * DS], F32, tag="t_if")
                    nc.scalar.activation(t_if[:], g_ps[:, 0:2 * DS], AF.Tanh, scale=0.5)
                    t_g = wpool.tile([B, DS], F32, tag="t_g")
                    nc.scalar.activation(t_g[:], g_ps[:, 2 * DS:3 * DS], AF.Tanh)
                    t_o = wpool.tile([B, DS], F32, tag="t_o")
                    nc.scalar.activation(t_o[:], g_ps[:, 3 * DS:4 * DS], AF.Tanh, scale=0.5)

                    i_s = wpool.tile([B, DS], F32, tag="i_s")
                    nc.vector.tensor_scalar(i_s[:], t_if[:, 0:DS], 0.5, 0.5, OP.mult, OP.add)
                    f_s = wpool.tile([B, DS], F32, tag="f_s")
                    nc.vector.tensor_scalar(f_s[:], t_if[:, DS:2 * DS], 0.5, 0.5, OP.mult, OP.add)
                    o_s = wpool.tile([B, DS], F32, tag="o_s")
                    nc.vector.tensor_scalar(o_s[:], t_o[:], 0.5, 0.5, OP.mult, OP.add)

                    tm1 = wpool.tile([B, DS], F32, tag="tm1")
                    nc.vector.tensor_mul(tm1[:], f_s[:], c_t[:])
                    tm2 = wpool.tile([B, DS], F32, tag="tm2")
                    nc.vector.tensor_mul(tm2[:], i_s[:], t_g[:])
                    nc.vector.tensor_add(c_t[:], tm1[:], tm2[:])
                    tc2 = wpool.tile([B, DS], F32, tag="tc2")
                    nc.scalar.activation(tc2[:], c_t[:], AF.Tanh)
                    h2l = wpool.tile([B, DS], BF16, tag="h2l")
                    nc.vector.tensor_mul(h2l[:], o_s[:], tc2[:])

                    # ---- h exchange (selector matmul + AllReduce) ----
                    ht_ps = ps_ht.tile([DS, B], BF16, tag="ht")
                    nc.tensor.transpose(ht_ps[:], h2l[:], identb[:])
                    h2Tl = wpool.tile([DS, B], BF16, tag="h2Tl")
                    nc.vector.tensor_copy(h2Tl[:], ht_ps[:])
                    sel_ps = ps_sel.tile([B, D], F32, tag="sel")
                    nc.tensor.matmul(sel_ps[:], h2Tl[:], sel[:], start=True, stop=True)
                    x2 = wpool.tile([B, D], BF16, tag="x2")
                    nc.vector.tensor_copy(x2[:], sel_ps[:])
                    xbi = dram.tile([B, D], BF16, tag="h_i")
                    xbo = dram.tile([B, D], BF16, tag="h_o")
                    nc.sync.dma_start(xbi[:], x2[:])
                    nc.gpsimd.collective_compute(
                        "AllReduce", OP.add, replica_groups=RG, ins=[xbi[:]], outs=[xbo[:]]
                    )
                    nc.sync.dma_start(h2full[:], xbo[:])

                # tail: preds for last step
                transposes_and_fc(T - 1)

    nc.compile()
    return nc


def _prepare_inputs(T, features, captions, embedding, Wa_w, Wa_b, Ua_w, Ua_b,
                    Va_w, Va_b, W_ih, W_hh, b_ih, b_hh, ih_w, ih_b, ic_w, ic_b,
                    fc_w, fc_b):
    caps = np.asarray(captions)[:, :T]
    embs = np.asarray(embedding)[caps]                      # [B, T, EMB]
    embT4 = _bf(embs.transpose(2, 1, 0).reshape(4, 128, T, B).transpose(1, 0, 2, 3))

    featT = _bf(np.asarray(features).transpose(2, 1, 0))    # [E, N, B]
    feat_e = np.asarray(features).transpose(0, 2, 1)        # [B, E, N]

    W_icT = np.asarray(W_ih)[:, :E].T                       # [E, 4D]
    W_ieT = np.asarray(W_ih)[:, E:].T                       # [EMB, 4D]
    WaT = np.asarray(Wa_w).T                                # [E, D]
    UaTf = np.asarray(Ua_w).T                               # [D, D]
    WhhTf = np.asarray(W_hh).T                              # [D, 4D]
    bsum = np.asarray(b_ih) + np.asarray(b_hh)
    ihicT = np.concatenate([np.asarray(ih_w) / N, np.asarray(ic_w) / N], axis=0).T  # [E, 2D]
    ihicb = np.concatenate([np.asarray(ih_b), np.asarray(ic_b)])

    in_maps = []
    for j in range(NC):
        ds = np.arange(j * DS, (j + 1) * DS)
        gs = (np.arange(4)[:, None] * D + ds[None, :]).reshape(-1)  # [GS]
        es = np.arange(j * 256, (j + 1) * 256)
        vs = np.arange(j * VS, (j + 1) * VS)

        rhs_gfp = np.concatenate([W_icT[:, gs], WaT[:, ds]], axis=1)  # [E, GS+DS]
        selm = np.zeros((DS, D), np.float32)
        selm[np.arange(DS), ds] = 1.0
        c0idx = np.empty((128, DS // 16), np.uint16)
        for p in range(128):
            for s in range(DS // 16):
                c0idx[p, s] = D + j * DS + s * 16 + (p % 16)

        in_maps.append({
            "featT": featT,
            "feat_en": _bf(feat_e[:, es, :]),
            "embT4": embT4,
            "rhs_gfp": _bf(rhs_gfp.reshape(EK, 128, GS + DS).transpose(1, 0, 2)),
            "WieT": _bf(W_ieT[:, gs].reshape(4, 128, GS).transpose(1, 0, 2)),
            "WhhT": _bf(WhhTf[:, gs].reshape(4, 128, GS).transpose(1, 0, 2)),
            "UaT": _bf(UaTf[:, ds].reshape(4, 128, DS).transpose(1, 0, 2)),
            "fcT": _bf(np.asarray(fc_w)[vs].T.reshape(4, 128, VS).transpose(1, 0, 2)),
            "ihicT": _bf(ihicT[es].reshape(2, 128, 2 * D).transpose(1, 0, 2)),
            "ihicb": _bc(ihicb).astype(np.float32),
            "biasg": _bc(bsum[gs]).astype(np.float32),
            "wuab": _bc((np.asarray(Wa_b) + np.asarray(Ua_b))[ds]).astype(np.float32),
            "vab": _bf(_bc(np.asarray(Va_w)[0, ds])),
            "sel": _bf(selm),
            "c0idx": c0idx,
            "fcb": _bc(np.asarray(fc_b)[vs]).astype(np.float32),
        })
    return in_maps


def kernel(**inputs):
    captions = np.asarray(inputs["captions"])
    T = captions.shape[1] - 1
    if T not in _CACHE:
        _CACHE[T] = build_program(T)
    nc = _CACHE[T]
    in_maps = _prepare_inputs(T, **inputs)
    res = run_bass_kernel_spmd(nc, in_maps, core_ids=list(range(NC)))
    att = np.asarray(res.results[0]["att"], np.float32)
    preds = np.concatenate(
        [np.asarray(res.results[j]["preds"], np.float32) for j in range(NC)], axis=-1
    )
    return att, preds
